# revision 2
# baseline (speedup 1.0000x reference)
"""HardNegativeInfoNCELoss on 8 Trainium2 NeuronCores.

Strategy:
  * Host: L2-normalize anchor/positive/negative_pool (fp32), transpose so the
    contraction dim (D=256) is the partition dim, shard pool columns across 8
    cores (M/8 = 32768 each).
  * Device (SPMD, per core): stream the pool shard once; for every
    (128-anchor tile, 2048-pool-column chunk) accumulate the fp32 sim tile in
    PSUM via 8 matmuls (2 K-halves x 4 N-slices of 512) and reduce it with a
    single DVE max8 -> top-8 sim values per anchor per chunk.
  * Host: merge the 8x16x8 = 1024 candidate values per anchor, take the top-10
    (a global top-10 member can only be missed if >=9 of them landed in one
    2048-column chunk - probability ~1e-14), append the positive logit and
    evaluate the InfoNCE loss.
"""

import os
import sys

import numpy as np


def _ensure_concourse():
    try:
        import concourse  # noqa: F401
        return
    except ImportError:
        pass
    for p in ("/opt/trn_rl_repo", "/root/.axon_site/_ro/trn_rl_repo"):
        if os.path.isdir(os.path.join(p, "concourse")):
            sys.path.insert(0, p)
            return


_ensure_concourse()

N_CORES = 8
B = 1024
D = 256
M = 262144
M_SHARD = M // N_CORES  # 32768
CHUNK = 2048  # sim columns handled per PSUM group (4 banks)
N_CHUNKS = M_SHARD // CHUNK  # 16
NB = B // 128  # 8 anchor tiles
NJ = CHUNK // 512  # 4 matmuls of N=512 per K-half
CAND_COLS = NB * N_CHUNKS * 8  # 1024
TEMPERATURE = 0.07
NUM_HARD_NEGATIVES = 10
EPS = 1e-12

_program = None


def _build_program():
    import concourse.bacc as bacc
    import concourse.mybir as mybir
    from concourse.tile import TileContext

    nc = bacc.Bacc(
        "TRN2", target_bir_lowering=False, debug=False, num_devices=N_CORES
    )
    AT = nc.dram_tensor("AT", [D, B], mybir.dt.float32, kind="ExternalInput")
    PT = nc.dram_tensor("PT", [D, M_SHARD], mybir.dt.float32, kind="ExternalInput")
    CAND = nc.dram_tensor(
        "CAND", [128, CAND_COLS], mybir.dt.float32, kind="ExternalOutput"
    )

    f32 = mybir.dt.float32
    with TileContext(nc) as tc:
        with (
            tc.tile_pool(name="const", bufs=1) as cpool,
            tc.tile_pool(name="stream", bufs=2) as spool,
            tc.tile_pool(name="psum", bufs=2, space="PSUM") as ppool,
        ):
            at0 = cpool.tile([128, B], f32)
            at1 = cpool.tile([128, B], f32)
            nc.sync.dma_start(out=at0, in_=AT[0:128, :])
            nc.sync.dma_start(out=at1, in_=AT[128:256, :])
            cand = cpool.tile([128, CAND_COLS], f32)

            for c in range(N_CHUNKS):
                cs = slice(c * CHUNK, (c + 1) * CHUNK)
                pt0 = spool.tile([128, CHUNK], f32, tag="pt0", name="pt0")
                pt1 = spool.tile([128, CHUNK], f32, tag="pt1", name="pt1")
                nc.sync.dma_start(out=pt0, in_=PT[0:128, cs])
                nc.sync.dma_start(out=pt1, in_=PT[128:256, cs])
                for b in range(NB):
                    bs = slice(b * 128, (b + 1) * 128)
                    ps = ppool.tile([128, CHUNK], f32, tag="ps", name="ps")
                    for j in range(NJ):
                        js = slice(j * 512, (j + 1) * 512)
                        nc.tensor.matmul(
                            ps[:, js], at0[:, bs], pt0[:, js], start=True, stop=False
                        )
                        nc.tensor.matmul(
                            ps[:, js], at1[:, bs], pt1[:, js], start=False, stop=True
                        )
                    o = (b * N_CHUNKS + c) * 8
                    nc.vector.max(out=cand[:, o : o + 8], in_=ps)
            nc.sync.dma_start(out=CAND[:, :], in_=cand)
    nc.compile()
    return nc


def _get_program():
    global _program
    if _program is None:
        _program = _build_program()
    return _program


def _normalize_rows(x):
    n = np.sqrt((x.astype(np.float32) ** 2).sum(axis=-1, keepdims=True))
    return x / np.maximum(n, EPS)


def run_device(anchor, negative_pool, trace=False, tmpdir=None):
    """Run the SPMD device program; returns (per-core CAND list, results obj)."""
    from concourse.bass_utils import run_bass_kernel_spmd

    a = _normalize_rows(np.asarray(anchor, dtype=np.float32))
    n = _normalize_rows(np.asarray(negative_pool, dtype=np.float32))
    at = np.ascontiguousarray(a.T)  # [D, B]
    nt = n.T  # [D, M] view
    in_maps = []
    for c in range(N_CORES):
        pt = np.ascontiguousarray(nt[:, c * M_SHARD : (c + 1) * M_SHARD])
        in_maps.append({"AT": at, "PT": pt})
    nc = _get_program()
    res = run_bass_kernel_spmd(
        nc, in_maps, core_ids=list(range(N_CORES)), trace=trace, tmpdir=tmpdir
    )
    cands = [res.results[c]["CAND"] for c in range(N_CORES)]
    return cands, res


def merge_loss(anchor, positive, cands):
    a = _normalize_rows(np.asarray(anchor, dtype=np.float32))
    p = _normalize_rows(np.asarray(positive, dtype=np.float32))
    pos_sim = (a * p).sum(axis=-1, dtype=np.float32) / TEMPERATURE  # [B]

    # cand[core][p_row, (b*N_CHUNKS + c)*8 + k] -> anchor b*128+p_row
    per_core = [
        cand.reshape(128, NB, N_CHUNKS * 8).transpose(1, 0, 2) for cand in cands
    ]  # [NB, 128, 128] each
    allc = np.concatenate(per_core, axis=-1).reshape(B, -1)  # [B, 1024]
    part = np.partition(allc, allc.shape[1] - NUM_HARD_NEGATIVES, axis=1)[
        :, -NUM_HARD_NEGATIVES:
    ]
    hard = np.sort(part, axis=1)[:, ::-1] / TEMPERATURE  # [B, 10] descending

    logits = np.concatenate([pos_sim[:, None], hard], axis=1).astype(np.float64)
    mx = logits.max(axis=1, keepdims=True)
    lse = mx[:, 0] + np.log(np.exp(logits - mx).sum(axis=1))
    loss = -(logits[:, 0] - lse).mean()
    return np.float32(loss)


def kernel(anchor, positive, negative_pool):
    cands, _ = run_device(anchor, negative_pool)
    return np.asarray(merge_loss(anchor, positive, cands), dtype=np.float32)


# revision 5
# speedup vs baseline: 2.9632x; 2.9632x over previous
"""HardNegativeInfoNCELoss on 8 Trainium2 NeuronCores.

Strategy:
  * Host: L2-normalize anchor/positive/negative_pool (fp32), transpose so the
    contraction dim (D=256) is the partition dim, shard pool columns across 8
    cores (M/8 = 32768 each).
  * Device (SPMD, per core): stream the pool shard once; for every
    (128-anchor tile, 2048-pool-column chunk) accumulate the fp32 sim tile in
    PSUM via 8 matmuls (2 K-halves x 4 N-slices of 512) and reduce it with a
    single DVE max8 -> top-8 sim values per anchor per chunk.
  * Host: merge the 8x16x8 = 1024 candidate values per anchor, take the top-10
    (a global top-10 member can only be missed if >=9 of them landed in one
    2048-column chunk - probability ~1e-14), append the positive logit and
    evaluate the InfoNCE loss.
"""

import os
import sys

import numpy as np


def _ensure_concourse():
    try:
        import concourse  # noqa: F401
        return
    except ImportError:
        pass
    for p in ("/opt/trn_rl_repo", "/root/.axon_site/_ro/trn_rl_repo"):
        if os.path.isdir(os.path.join(p, "concourse")):
            sys.path.insert(0, p)
            return


_ensure_concourse()

N_CORES = 8
B = 1024
D = 256
M = 262144
M_SHARD = M // N_CORES  # 32768
CHUNK = 2048  # sim columns handled per PSUM group (4 banks)
N_CHUNKS = M_SHARD // CHUNK  # 16
NB = B // 128  # 8 anchor tiles
NJ = CHUNK // 512  # 4 matmuls of N=512 per K-half
CAND_COLS = NB * N_CHUNKS * 8  # 1024
TEMPERATURE = 0.07
NUM_HARD_NEGATIVES = 10
EPS = 1e-12

_program = None


def _build_program():
    import concourse.bacc as bacc
    import concourse.mybir as mybir
    from concourse.tile import TileContext

    nc = bacc.Bacc(
        "TRN2", target_bir_lowering=False, debug=False, num_devices=N_CORES
    )
    bf16 = mybir.dt.bfloat16
    f32 = mybir.dt.float32
    AT = nc.dram_tensor("AT", [D, B], bf16, kind="ExternalInput")
    PT = nc.dram_tensor("PT", [D, M_SHARD], bf16, kind="ExternalInput")
    CAND = nc.dram_tensor(
        "CAND", [128, CAND_COLS], f32, kind="ExternalOutput"
    )

    MMN = 512  # moving free dim per matmul (PSUM bank = 512 fp32)
    with TileContext(nc) as tc:
        with (
            tc.tile_pool(name="const", bufs=1) as cpool,
            tc.tile_pool(name="stream", bufs=2) as spool,
            tc.tile_pool(name="psum", bufs=2, space="PSUM") as ppool,
        ):
            at0 = cpool.tile([128, B], bf16)
            at1 = cpool.tile([128, B], bf16)
            nc.sync.dma_start(out=at0, in_=AT[0:128, :])
            nc.sync.dma_start(out=at1, in_=AT[128:256, :])
            cand = cpool.tile([128, CAND_COLS], f32)

            for c in range(N_CHUNKS):
                cs = slice(c * CHUNK, (c + 1) * CHUNK)
                pt0 = spool.tile([128, CHUNK], bf16, tag="pt0", name="pt0")
                pt1 = spool.tile([128, CHUNK], bf16, tag="pt1", name="pt1")
                nc.sync.dma_start(out=pt0, in_=PT[0:128, cs])
                nc.sync.dma_start(out=pt1, in_=PT[128:256, cs])
                for b in range(NB):
                    bs = slice(b * 128, (b + 1) * 128)
                    ps = ppool.tile([128, CHUNK], f32, tag="ps", name="ps")
                    # d0 for both N-slices first so each LDWEIGHTS serves two
                    # matmuls, then d1 closes the accumulation groups.
                    for j in range(CHUNK // MMN):
                        js = slice(j * MMN, (j + 1) * MMN)
                        nc.tensor.matmul(
                            ps[:, js], at0[:, bs], pt0[:, js], start=True, stop=False
                        )
                    for j in range(CHUNK // MMN):
                        js = slice(j * MMN, (j + 1) * MMN)
                        nc.tensor.matmul(
                            ps[:, js], at1[:, bs], pt1[:, js], start=False, stop=True
                        )
                    o = (b * N_CHUNKS + c) * 8
                    nc.vector.max(out=cand[:, o : o + 8], in_=ps)
            nc.sync.dma_start(out=CAND[:, :], in_=cand)
    nc.compile()
    return nc


def _get_program():
    global _program
    if _program is None:
        _program = _build_program()
    return _program


def _normalize_rows(x):
    n = np.sqrt((x.astype(np.float32) ** 2).sum(axis=-1, keepdims=True))
    return x / np.maximum(n, EPS)


def run_device(anchor, negative_pool, trace=False, tmpdir=None):
    """Run the SPMD device program; returns (per-core CAND list, results obj)."""
    from concourse.bass_utils import run_bass_kernel_spmd

    import ml_dtypes

    bf16 = ml_dtypes.bfloat16
    a = _normalize_rows(np.asarray(anchor, dtype=np.float32))
    n = _normalize_rows(np.asarray(negative_pool, dtype=np.float32))
    at = np.ascontiguousarray(a.T).astype(bf16)  # [D, B]
    nt = n.T.astype(bf16)  # [D, M]
    in_maps = []
    for c in range(N_CORES):
        pt = np.ascontiguousarray(nt[:, c * M_SHARD : (c + 1) * M_SHARD])
        in_maps.append({"AT": at, "PT": pt})
    nc = _get_program()
    res = run_bass_kernel_spmd(
        nc, in_maps, core_ids=list(range(N_CORES)), trace=trace, tmpdir=tmpdir
    )
    cands = [res.results[c]["CAND"] for c in range(N_CORES)]
    return cands, res


def merge_loss(anchor, positive, cands):
    a = _normalize_rows(np.asarray(anchor, dtype=np.float32))
    p = _normalize_rows(np.asarray(positive, dtype=np.float32))
    pos_sim = (a * p).sum(axis=-1, dtype=np.float32) / TEMPERATURE  # [B]

    # cand[core][p_row, (b*N_CHUNKS + c)*8 + k] -> anchor b*128+p_row
    per_core = [
        cand.reshape(128, NB, N_CHUNKS * 8).transpose(1, 0, 2) for cand in cands
    ]  # [NB, 128, 128] each
    allc = np.concatenate(per_core, axis=-1).reshape(B, -1)  # [B, 1024]
    part = np.partition(allc, allc.shape[1] - NUM_HARD_NEGATIVES, axis=1)[
        :, -NUM_HARD_NEGATIVES:
    ]
    hard = np.sort(part, axis=1)[:, ::-1] / TEMPERATURE  # [B, 10] descending

    logits = np.concatenate([pos_sim[:, None], hard], axis=1).astype(np.float64)
    mx = logits.max(axis=1, keepdims=True)
    lse = mx[:, 0] + np.log(np.exp(logits - mx).sum(axis=1))
    loss = -(logits[:, 0] - lse).mean()
    return np.float32(loss)


def kernel(anchor, positive, negative_pool):
    cands, _ = run_device(anchor, negative_pool)
    return np.asarray(merge_loss(anchor, positive, cands), dtype=np.float32)
